# revision 1
# baseline (speedup 1.0000x reference)
"""AttnBlock (GroupNorm + single-head self-attention + residual) on 8 trn2 cores.

Problem: X [4, 512, 64, 64] f32. Per batch element: GroupNorm(32 groups), then
1x1-conv Q/K/V projections, softmax attention over n=h*w=4096 positions,
proj_out, residual add.

Sharding: 8 cores = 4 batch elements x 2 query-halves. Each core computes the
full GroupNorm + K/V for its batch element (duplicated within the pair) and
attention output for its 2048-query half.

Layout strategy (per core):
  Hn, K, Q kept channel-major [c, n] (c on partitions)  -> projections are
  natural matmuls.  S^T[k, q] = sum_c K[c,k] Q[c,q] computed with k on
  partitions so softmax sums reduce via a ones-vector matmul on the PE and
  Ho[q, c] = sum_k expS[k,q] V[k,c] accumulates flash-style in PSUM without
  ever materializing/transposing the 4096x4096 attention matrix.
  Softmax skips max-subtraction: |S*scale| < ~10 here, exp is safe in f32.

All big matmuls run in float32r (full PE rate at N=512, ~1.5e-4 rel err).

SBUF (208KB/partition) forces a two-pass GroupNorm: pass 1 streams X for
stats only; pass 2 re-reads X in halves, normalizes, and immediately
projects K (staged to DRAM scratch) and V.  Q likewise from the Xq input.
K is reloaded into SBUF for the attention phase once Hn is gone.
"""

import numpy as np

B, C, H, W = 4, 512, 64, 64
N = H * W            # 4096 keys per batch element
NQ = N // 2          # 2048 queries per core
CT = C // 128        # 4 channel tiles
NT = N // 128        # 32 key tiles
QC = NQ // 512       # 4 query chunks of 512
GROUPS = 32
GPT = GROUPS // CT   # 8 groups per 128-channel tile
GSZ = C // GROUPS    # 16 channels per group
EPS = 1e-5
SCALE = float(C) ** -0.5

_CACHE = {}


def _build(debug=False):
    from contextlib import ExitStack
    from concourse import bacc
    import concourse.mybir as mybir
    import concourse.tile as tile
    from concourse.masks import make_identity

    f32 = mybir.dt.float32
    f32r = mybir.dt.float32r
    AF = mybir.ActivationFunctionType
    OP = mybir.AluOpType

    nc = bacc.Bacc()
    X = nc.dram_tensor("X", [C, N], f32, kind="ExternalInput")
    Xq = nc.dram_tensor("Xq", [C, NQ], f32, kind="ExternalInput")
    wT = {
        nm: nc.dram_tensor(nm, [C, C], f32, kind="ExternalInput")
        for nm in ("wqT", "wkT", "wvT", "wpT")
    }
    vecs = {
        nm: nc.dram_tensor(nm, [C], f32, kind="ExternalInput")
        for nm in ("bq", "bk", "bpe", "gn_w", "gn_b")
    }
    gmat_d = nc.dram_tensor("gmat_d", [128, GPT], f32, kind="ExternalInput")
    ones2_d = nc.dram_tensor("ones2_d", [128, 2], f32, kind="ExternalInput")
    gmatT_d = nc.dram_tensor("gmatT_d", [GPT, 128], f32, kind="ExternalInput")
    out = nc.dram_tensor("out", [C, NQ], f32, kind="ExternalOutput")
    dbg = {}
    if debug:
        for nm, shp in [("dbg_scbi", [128, 2 * CT]), ("dbg_q", [128, 512]),
                        ("dbg_k", [128, 512]), ("dbg_v", [128, C]),
                        ("dbg_es", [128, 512]), ("dbg_sums", [128, 8]),
                        ("dbg_ho", [128, 512]), ("dbg_hoT", [128, 512]),
                        ("dbg_sraw", [128, 512])]:
            dbg[nm] = nc.dram_tensor(nm, shp, f32, kind="ExternalOutput")

    def col(v, ci):
        # [C] dram vector -> [128, 1] AP for channel tile ci
        return vecs[v][ci * 128:(ci + 1) * 128].rearrange("(p one) -> p one", one=1)

    def load_f32r(pool, stage_pool, dram_ap, shape, tag):
        """DMA f32 -> staging, DVE-convert -> f32r tile (real format change)."""
        st = stage_pool.tile(shape, f32, tag="ld_stage", name="ld_stage")
        nc.sync.dma_start(out=st, in_=dram_ap)
        t = pool.tile(shape, f32r, tag=tag, name=tag)
        nc.vector.tensor_copy(out=t, in_=st)
        return t

    # fp32r is an opaque on-chip format: every fp32r operand must be produced
    # by a compute-engine conversion (DVE copy), never by a bitcast DMA.

    with tile.TileContext(nc) as tc, ExitStack() as ctx:
        consts = ctx.enter_context(tc.tile_pool(name="consts", bufs=1))
        pp_acc = ctx.enter_context(tc.tile_pool(name="pp_acc", bufs=4, space="PSUM"))
        pp_sps = ctx.enter_context(tc.tile_pool(name="pp_sps", bufs=3, space="PSUM"))
        pp_sums = ctx.enter_context(tc.tile_pool(name="pp_sums", bufs=1, space="PSUM"))

        # ---- pass A: stream X quarters for GroupNorm statistics ----
        # (emitted FIRST so the X DMA triggers lead the queues)
        gst_cm = tc.tile_pool(name="gn_stats", bufs=2)
        gstats = gst_cm.__enter__()
        xst_cm = tc.tile_pool(name="xstream", bufs=3)
        xstream = xst_cm.__enter__()
        if True:
            rowst_all = gstats.tile([128, CT, 2], f32r, tag="rowst", name="rowst")
            with nc.named_scope("gn"):
                for ci in range(CT):
                    stats = gstats.tile([128, N // 512, 6], f32, tag="bnst",
                                        name="bnst")
                    for q4 in range(4):
                        xs = xstream.tile([128, N // 4], f32, tag="xs", name="xs")
                        eng = nc.gpsimd if (ci * 4 + q4) % 2 else nc.sync
                        eng.dma_start(
                            out=xs,
                            in_=X[ci * 128:(ci + 1) * 128,
                                  q4 * (N // 4):(q4 + 1) * (N // 4)])
                        for s in range(N // 4 // 512):
                            nc.vector.bn_stats(
                                out=stats[:, q4 * 2 + s, :],
                                in_=xs[:, s * 512:(s + 1) * 512])
                    mv = gstats.tile([128, 2], f32, tag="mv", name="mv")
                    nc.vector.bn_aggr(out=mv, in_=stats)
                    # rowstats = [mean, E[x^2]] ; E[x^2] = var + mean^2
                    nc.vector.tensor_copy(out=rowst_all[:, ci, 0:1],
                                          in_=mv[:, 0:1])
                    m2 = gstats.tile([128, 1], f32, tag="m2", name="m2")
                    nc.vector.tensor_mul(out=m2, in0=mv[:, 0:1], in1=mv[:, 0:1])
                    nc.vector.tensor_add(out=rowst_all[:, ci, 1:2],
                                         in0=mv[:, 1:2], in1=m2)


        # ---- constants ----
        ident = consts.tile([128, 128], f32, tag="ident", name="ident")
        make_identity(nc, ident)
        with tc.tile_pool(name="cstage", bufs=2) as cstage:
            gmat = load_f32r(consts, cstage, gmat_d[:, :], [128, GPT], "gmat")
            gmatT = load_f32r(consts, cstage, gmatT_d[:, :], [GPT, 128], "gmatT")
            ones_col = load_f32r(consts, cstage, ones2_d[:, :], [128, 2], "ones")
        eps_t = consts.tile([128, 1], f32, tag="eps", name="eps")
        nc.vector.memset(eps_t, EPS)
        vt = {}
        for nm in ("bq", "bk", "bpe", "gn_w", "gn_b"):
            vt[nm] = consts.tile([128, CT], f32, tag=nm, name=nm)
            nc.sync.dma_start(
                out=vt[nm], in_=vecs[nm].rearrange("(c p) -> p c", p=128))
        # per-row GN affine: hn = x * sc_all[:,ci] + bi_all[:,ci]
        sc_all = consts.tile([128, CT], f32, tag="sc_all", name="sc_all")
        bi_all = consts.tile([128, CT], f32, tag="bi_all", name="bi_all")
        # proj weights stay resident (needed at the very end)
        wpT_sb = []
        with tc.tile_pool(name="wstage", bufs=2) as wstage:
            for ci in range(CT):
                wpT_sb.append(load_f32r(
                    consts, wstage, wT["wpT"][ci * 128:(ci + 1) * 128, :],
                    [128, C], f"wpT{ci}"))

        q_sb = [consts.tile([128, NQ], f32r, tag=f"q{co}", name=f"q{co}")
                for co in range(CT)]
        v_sb = [consts.tile([128, C], f32r, tag=f"v{nt}", name=f"v{nt}")
                for nt in range(NT)]

        # GN is folded into the projections: K = (wk*sc) @ X + (wk@bi + bk),
        # V likewise with its bias routed through proj_out (softmax rows sum
        # to 1), Q likewise.  X itself only needs a format conversion (on the
        # otherwise-idle Scalar engine) and the stats chain gates only the
        # small weight-fold ops, not a full normalization pass over X.
        bi2 = consts.tile([128, CT, 2], f32r, tag="bi2", name="bi2")
        kb_sb = consts.tile([128, CT], f32, tag="kb_sb", name="kb_sb")
        qb_sb = consts.tile([128, CT], f32, tag="qb_sb", name="qb_sb")
        vb2 = consts.tile([128, CT, 2], f32r, tag="vb2", name="vb2")
        pbe = consts.tile([128, CT], f32, tag="pbe", name="pbe")


        with nc.named_scope("gn2"):
                # group-reduce 128 rows -> 8 groups -> broadcast, all ci at once
                gps = pp_sps.tile([GPT, CT, 2], f32, tag="s_ps", name="gps")
                nc.tensor.matmul(out=gps, lhsT=gmat,
                                 rhs=rowst_all.rearrange("p c two -> p (c two)"),
                                 start=True, stop=True)
                gsb = gstats.tile([GPT, CT * 2], f32r, tag="gsb", name="gsb")
                nc.vector.tensor_copy(out=gsb,
                                      in_=gps.rearrange("g c two -> g (c two)"))
                bps = pp_sps.tile([128, CT, 2], f32, tag="s_ps", name="bps")
                nc.tensor.matmul(out=bps, lhsT=gmatT, rhs=gsb,
                                 start=True, stop=True)
                gstat = gstats.tile([128, CT, 2], f32, tag="gstat", name="gstat")
                nc.scalar.mul(out=gstat, in_=bps, mul=1.0 / GSZ)

                means = gstat[:, :, 0:1].rearrange("p c one -> p (c one)")
                m2s = gstat[:, :, 1:2].rearrange("p c one -> p (c one)")
                var = gstats.tile([128, CT], f32, tag="var", name="var")
                mm_ = gstats.tile([128, CT], f32, tag="mm_", name="mm_")
                nc.vector.tensor_mul(out=mm_, in0=means, in1=means)
                nc.vector.tensor_sub(out=var, in0=m2s, in1=mm_)
                # rstd = 1/sqrt(var + eps)
                nc.scalar.activation(out=var, in_=var, func=AF.Sqrt,
                                     bias=eps_t, scale=1.0)
                rstd = gstats.tile([128, CT], f32, tag="rstd", name="rstd")
                nc.vector.reciprocal(out=rstd, in_=var)
                # sc = rstd * gn_w ; bi = gn_b - mean * sc
                nc.vector.tensor_mul(out=sc_all, in0=rstd, in1=vt["gn_w"])
                msc = gstats.tile([128, CT], f32, tag="msc", name="msc")
                nc.vector.tensor_mul(out=msc, in0=means, in1=sc_all)
                nc.vector.tensor_sub(out=bi_all, in0=vt["gn_b"], in1=msc)
                for ci in range(CT):
                    nc.vector.tensor_copy(
                        out=bi2[:, ci, :],
                        in_=bi_all[:, ci:ci + 1].to_broadcast((128, 2)))

        xst_cm.__exit__(None, None, None)
        gst_cm.__exit__(None, None, None)


        def bias_matvec(w_sb, rhs2, add_vec):
            """[128, CT] per-partition vector = w.T-chunks @ rhs2 (+add_vec)."""
            outt = consts.tile([128, CT], f32, tag=f"bv_{w_sb[0].tensor.name}",
                               name="bv")
            for co in range(CT):
                ps = pp_sps.tile([128, 2], f32, tag="s_ps", name="bv_ps")
                for ci in range(CT):
                    nc.tensor.matmul(
                        out=ps, lhsT=w_sb[ci][:, co * 128:(co + 1) * 128],
                        rhs=rhs2[:, ci, :],
                        start=(ci == 0), stop=(ci == CT - 1))
                if add_vec is not None:
                    nc.vector.tensor_add(out=outt[:, co:co + 1],
                                         in0=ps[:, 0:1],
                                         in1=add_vec[:, co:co + 1])
                else:
                    nc.vector.tensor_copy(out=outt[:, co:co + 1], in_=ps[:, 0:1])
            return outt

        def fold(w_sb):
            for ci in range(CT):
                nc.vector.tensor_scalar_mul(out=w_sb[ci], in0=w_sb[ci],
                                            scalar1=sc_all[:, ci:ci + 1])

        # K lives in SBUF from projection straight through attention.
        kpool = ctx.enter_context(tc.tile_pool(name="kpool", bufs=1))
        k_sb = [kpool.tile([128, N], f32r, tag=f"k{ci}", name=f"k{ci}")
                for ci in range(CT)]

        # ---- K/V/Q weight loads, bias matvecs, folds (overlap Q below) ----
        wkv_cm = tc.tile_pool(name="wkv", bufs=1)
        wkv = wkv_cm.__enter__()
        wk_sb, wv_sb = [], []
        for ci in range(CT):
            wk_sb.append(load_f32r(
                wkv, wkv, wT["wkT"][ci * 128:(ci + 1) * 128, :],
                [128, C], f"wk{ci}"))
            wv_sb.append(load_f32r(
                wkv, wkv, wT["wvT"][ci * 128:(ci + 1) * 128, :],
                [128, C], f"wv{ci}"))
        kb = bias_matvec(wk_sb, bi2, vt["bk"])
        nc.vector.tensor_copy(out=kb_sb, in_=kb)
        vb = bias_matvec(wv_sb, bi2, None)
        for ci in range(CT):
            nc.vector.tensor_copy(
                out=vb2[:, ci, :],
                in_=vb[:, ci:ci + 1].to_broadcast((128, 2)))
        pb = bias_matvec(wpT_sb, vb2, vt["bpe"])
        nc.vector.tensor_copy(out=pbe, in_=pb)
        fold(wk_sb)
        fold(wv_sb)

        # ---- Q (streamed Xq quarters) ----
        with tc.tile_pool(name="wq", bufs=1) as wqp:
            wq_sb = []
            for ci in range(CT):
                wq_sb.append(load_f32r(
                    wqp, wqp, wT["wqT"][ci * 128:(ci + 1) * 128, :],
                    [128, C], f"wq{ci}"))
            qb = bias_matvec(wq_sb, bi2, vt["bq"])
            nc.vector.tensor_copy(out=qb_sb, in_=qb)
            fold(wq_sb)
            with tc.tile_pool(name="hq_q", bufs=1) as hqpool:
                for qn in range(QC):
                    hq = []
                    for ci in range(CT):
                        t = hqpool.tile([128, 512], f32r, tag=f"xq{ci}",
                                        name=f"xq{ci}")
                        nc.gpsimd.dma_start(
                            out=t,
                            in_=Xq[ci * 128:(ci + 1) * 128,
                                   qn * 512:(qn + 1) * 512].bitcast(f32r))
                        nc.scalar.activation(out=t, in_=t.bitcast(f32),
                                             func=AF.Copy)
                        hq.append(t)
                    with nc.named_scope("qproj"):
                        for co in range(CT):
                            ps = pp_sps.tile([128, 512], f32, tag="s_ps",
                                             name="q_ps")
                            for ci in range(CT):
                                nc.tensor.matmul(
                                    out=ps,
                                    lhsT=wq_sb[ci][:, co * 128:(co + 1) * 128],
                                    rhs=hq[ci],
                                    start=(ci == 0), stop=(ci == CT - 1))
                            nc.vector.tensor_scalar_add(
                                out=q_sb[co][:, qn * 512:(qn + 1) * 512],
                                in0=ps, scalar1=qb_sb[:, co:co + 1])

        # ---- pass B: stream X eighths, project K (into SBUF) and V ----
        with tc.tile_pool(name="xb", bufs=2) as xbp:
            for e8 in range(8):
                ns = slice(e8 * 512, (e8 + 1) * 512)
                xb = []
                for ci in range(CT):
                    t = xbp.tile([128, 512], f32r, tag=f"xb{ci}", name=f"xb{ci}")
                    nc.gpsimd.dma_start(
                        out=t, in_=X[ci * 128:(ci + 1) * 128, ns].bitcast(f32r))
                    nc.scalar.activation(out=t, in_=t.bitcast(f32), func=AF.Copy)
                    xb.append(t)
                with nc.named_scope("kproj"):
                    for co in range(CT):
                        ps = pp_sps.tile([128, 512], f32, tag="s_ps", name="k_ps")
                        for ci in range(CT):
                            nc.tensor.matmul(
                                out=ps, lhsT=wk_sb[ci][:, co * 128:(co + 1) * 128],
                                rhs=xb[ci],
                                start=(ci == 0), stop=(ci == CT - 1))
                        nc.vector.tensor_scalar_add(out=k_sb[co][:, ns], in0=ps,
                                                    scalar1=kb_sb[:, co:co + 1])
                with nc.named_scope("vproj"):
                    for nt4 in range(4):
                        nt = e8 * 4 + nt4
                        ps = pp_sps.tile([128, 512], f32, tag="s_ps", name="v_ps")
                        for ci in range(CT):
                            nc.tensor.matmul(
                                out=ps,
                                lhsT=xb[ci][:, nt4 * 128:(nt4 + 1) * 128],
                                rhs=wv_sb[ci],
                                start=(ci == 0), stop=(ci == CT - 1))
                        nc.vector.tensor_copy(out=v_sb[nt], in_=ps)

        wkv_cm.__exit__(None, None, None)

        if debug:
            dt_ = consts.tile([128, 2 * CT], f32, tag="dbg1", name="dbg1")
            nc.vector.tensor_copy(out=dt_[:, :CT], in_=sc_all)
            nc.vector.tensor_copy(out=dt_[:, CT:], in_=bi_all)
            nc.sync.dma_start(out=dbg["dbg_scbi"][:, :], in_=dt_)
            dq = consts.tile([128, 512], f32, tag="dbg_q", name="dbg_q")
            nc.vector.tensor_copy(out=dq, in_=q_sb[0][:, :512])
            nc.sync.dma_start(out=dbg["dbg_q"][:, :], in_=dq)
            dv = consts.tile([128, C], f32, tag="dbg_v", name="dbg_v")
            nc.vector.tensor_copy(out=dv, in_=v_sb[0])
            nc.sync.dma_start(out=dbg["dbg_v"][:, :], in_=dv)

        # ---- attention ----
        with tc.tile_pool(name="work", bufs=2) as work:
            if debug:
                dk = work.tile([128, 512], f32, tag="dbg_k", name="dbg_k", bufs=1)
                nc.vector.tensor_copy(out=dk, in_=k_sb[0][:, :512])
                nc.sync.dma_start(out=dbg["dbg_k"][:, :], in_=dk)

            for qc in range(QC):
                qs = slice(qc * 512, (qc + 1) * 512)
                ho_ps = [pp_acc.tile([128, 512], f32, tag="acc", name="acc")
                         for _ in range(4)]
                sums_ps = pp_sums.tile([128, 8], f32, tag="sums", name="sums")
                nc.vector.memset(sums_ps, 0.0)
                def s_exp(kt):
                    s_ps = pp_sps.tile([128, 512], f32, tag="s_ps", name="s_ps")
                    with nc.named_scope("attn_s"):
                        for ci in range(CT):
                            nc.tensor.matmul(
                                out=s_ps, lhsT=k_sb[ci][:, kt * 128:(kt + 1) * 128],
                                rhs=q_sb[ci][:, qs],
                                start=(ci == 0), stop=(ci == CT - 1))
                    es = work.tile([128, 512], f32r, tag="es", name="es",
                                   bufs=4 if debug else 6)
                    nc.scalar.activation(out=es, in_=s_ps, func=AF.Exp, scale=SCALE)
                    return es

                es_next = s_exp(0)
                for kt in range(NT):
                    es = es_next
                    if kt + 1 < NT:
                        es_next = s_exp(kt + 1)
                    with nc.named_scope("attn_ho"):
                        for j in range(4):
                            nc.tensor.matmul(
                                out=ho_ps[j], lhsT=es[:, j * 128:(j + 1) * 128],
                                rhs=v_sb[kt],
                                start=(kt == 0), stop=(kt == NT - 1))
                            nc.tensor.matmul(
                                out=sums_ps[:, 2 * j:2 * j + 2],
                                lhsT=es[:, j * 128:(j + 1) * 128], rhs=ones_col,
                                start=False, stop=(kt == NT - 1),
                                skip_group_check=True)

                inv = work.tile([128, 8], f32, tag="inv", name="inv")
                nc.vector.reciprocal(out=inv, in_=sums_ps)
                if debug and qc == 0:
                    nc.sync.dma_start(out=dbg["dbg_sums"][:, :], in_=inv)

                hoT = [work.tile([128, 512], f32r, tag="hoT", name="hoT", bufs=4 if debug else 5)
                       for _ in range(CT)]
                scope_tail = nc.enter_named_scope("attn_tail", False)
                for j in range(4):
                    ho_sb = work.tile([128, 512], f32, tag="ho_sb", name="ho_sb", bufs=1 if debug else 2)
                    nc.vector.tensor_scalar_mul(out=ho_sb, in0=ho_ps[j],
                                                scalar1=inv[:, 2 * j:2 * j + 1])
                    if debug and qc == 0 and j == 0:
                        nc.sync.dma_start(out=dbg["dbg_ho"][:, :], in_=ho_sb)
                    for ci in range(CT):
                        tp = pp_sps.tile([128, 128], f32, tag="s_ps", name="tp")
                        nc.tensor.transpose(tp, ho_sb[:, ci * 128:(ci + 1) * 128],
                                            ident)
                        nc.vector.tensor_copy(
                            out=hoT[ci][:, j * 128:(j + 1) * 128], in_=tp)

                if debug and qc == 0:
                    dht = work.tile([128, 512], f32, tag="dbg_hoT", name="dbg_hoT", bufs=1)
                    nc.vector.tensor_copy(out=dht, in_=hoT[0])
                    nc.sync.dma_start(out=dbg["dbg_hoT"][:, :], in_=dht)
                nc.leave_named_scope("attn_tail", scope_tail[0], False)
                for co in range(CT):
                    ps = pp_sps.tile([128, 512], f32, tag="s_ps", name="pr_ps")
                    for ci in range(CT):
                        nc.tensor.matmul(
                            out=ps, lhsT=wpT_sb[ci][:, co * 128:(co + 1) * 128],
                            rhs=hoT[ci],
                            start=(ci == 0), stop=(ci == CT - 1))
                    xr = work.tile([128, 512], f32, tag="xr", name="xr", bufs=1 if debug else 2)
                    nc.sync.dma_start(out=xr, in_=Xq[co * 128:(co + 1) * 128, qs])
                    ot = work.tile([128, 512], f32, tag="ot", name="ot", bufs=1 if debug else 2)
                    nc.vector.tensor_scalar_add(out=ot, in0=ps,
                                                scalar1=pbe[:, co:co + 1])
                    nc.vector.tensor_add(out=ot, in0=ot, in1=xr)
                    nc.sync.dma_start(out=out[co * 128:(co + 1) * 128, qs], in_=ot)

    nc.compile()
    return nc


def _get_nc():
    if "nc" not in _CACHE:
        _CACHE["nc"] = _build()
    return _CACHE["nc"]


def _prep_in_maps(X, gn_w, gn_b, wq, bq, wk, bk, wv, bv, wp, bp):
    X = np.ascontiguousarray(np.asarray(X, dtype=np.float32))
    f = lambda a: np.ascontiguousarray(np.asarray(a, dtype=np.float32))
    gn_w, gn_b, bq, bk, bv, bp = map(f, (gn_w, gn_b, bq, bk, bv, bp))
    wq, wk, wv, wp = map(f, (wq, wk, wv, wp))

    Xf = X.reshape(B, C, N)
    bpe = wp @ bv + bp  # bv folded through proj_out (sum_k softmax == 1)
    wqT = np.ascontiguousarray(wq.T)
    wkT = np.ascontiguousarray(wk.T)
    wvT = np.ascontiguousarray(wv.T)
    wpT = np.ascontiguousarray(wp.T)

    gmat = np.zeros((128, GPT), np.float32)
    for g in range(GPT):
        gmat[g * GSZ:(g + 1) * GSZ, g] = 1.0
    gmatT = np.ascontiguousarray(gmat.T)

    in_maps = []
    for core in range(8):
        bi, half = core // 2, core % 2
        q0 = half * NQ
        Xb = Xf[bi]
        in_maps.append({
            "X": Xb,
            "Xq": np.ascontiguousarray(Xb[:, q0:q0 + NQ]),
            "wqT": wqT, "wkT": wkT, "wvT": wvT, "wpT": wpT,
            "bq": bq, "bk": bk, "bpe": bpe, "gn_w": gn_w, "gn_b": gn_b,
            "gmat_d": gmat, "gmatT_d": gmatT,
            "ones2_d": np.ones((128, 2), np.float32),
        })
    return in_maps


_last_in_maps = None


def kernel(X, gn_w, gn_b, wq, bq, wk, bk, wv, bv, wp, bp):
    from concourse.bass_utils import run_bass_kernel_spmd

    global _last_in_maps
    in_maps = _prep_in_maps(X, gn_w, gn_b, wq, bq, wk, bk, wv, bv, wp, bp)
    _last_in_maps = in_maps
    nc = _get_nc()
    res = run_bass_kernel_spmd(nc, in_maps, list(range(8)))
    out = np.empty((B, C, N), np.float32)
    for core in range(8):
        bi, half = core // 2, core % 2
        out[bi][:, half * NQ:(half + 1) * NQ] = res.results[core]["out"]
    return out.reshape(B, C, H, W)



# revision 6
# speedup vs baseline: 1.5796x; 1.5796x over previous
"""AttnBlock (GroupNorm + single-head self-attention + residual) on 8 trn2 cores.

Problem: X [4, 512, 64, 64] f32. Per batch element: GroupNorm(32 groups), then
1x1-conv Q/K/V projections, softmax attention over n=h*w=4096 positions,
proj_out, residual add.  8 cores = 4 batch elements x 2 query-halves.

v2 strategy: fp8e4m3 DoubleRow matmuls (256-deep contraction per instruction,
2 rows/PE-cycle) for every large matmul, plus algebraic fusions that shrink
the graph:

  - proj_out is folded into the V projection on the HOST: wpv = wp @ wv, so
    the attention accumulator directly produces the (unnormalized) projected
    output. No HoT materialization, no separate P-proj matmuls, one less fp8
    rounding stage.
  - K's bias adds a per-query constant to the logits -> cancels in softmax.
    Dropped. V's bias is routed through proj_out (pbe = wp @ bv + bp, added
    to the residual tiles once). Softmax itself is unnormalized
    exp(S*scale - 3.5); the shift cancels in the final normalization and
    keeps es inside fp8e4's +-240 range with no max pass / NaN risk.
  - Host pre-quantizes X and weights to fp8e4m3 in DoubleRow pair layout
    [pair, part, 2, free]. For the half=1 core of each pair the two key
    halves of x8 are swapped so the kernel always treats columns 0..2047 as
    its queries (softmax is permutation-invariant over keys).
  - GroupNorm stats run on the fp8 X (bn_stats per chunk as DMAs land);
    sc/bi fold into the fp8 activation hn8 via one Pool-engine
    tensor_scalar pass (Pool has no PSUM port, so it gets all SBUF-only
    work; every PSUM->SBUF move runs on DVE or ACT).
  - Attention inner loop per key-tile pair: 4 DR matmuls for S^T[k,q] into
    a 2-bank PSUM tile, one ACT exp (psum->fp8 SBUF), 4 DR matmuls
    accumulating out_un[c,q] (4 banks), with es8 kept resident; row sums
    run as a post-pass of ones-lhsT DR matmuls into a recycled S-pool slot
    (all 128 output partitions identical -> no partition broadcast).
  - Tail per query chunk: reciprocal, out = out_un * inv + (X + pbe), DMA.
  - Junk DR matmuls tied to x8 DMA arrivals keep the PE HAM clock-gate warm
    through the prologue.

PSUM in attention: 2x2-bank S tiles + 4 accumulator banks = 8 exactly.
"""

import numpy as np
import ml_dtypes

B, C, H, W = 4, 512, 64, 64
N = H * W            # 4096 keys per batch element
NQ = N // 2          # 2048 queries per core
CT = C // 128        # 4 channel tiles
CP = CT // 2         # 2 channel-tile pairs (DoubleRow)
NT = N // 128        # 32 key tiles
NTP = NT // 2        # 16 key-tile pairs
QC = NQ // 512       # 4 query chunks of 512
NC8 = N // 512       # 8 key chunks of 512
GROUPS = 32
GPT = GROUPS // CT   # 8 groups per 128-channel tile
GSZ = C // GROUPS    # 16 channels per group
EPS = 1e-5
SCALE = float(C) ** -0.5
ESHIFT = -3.5

_CACHE = {}
F8NP = ml_dtypes.float8_e4m3


def _build():
    from contextlib import ExitStack
    from concourse import bacc
    import concourse.mybir as mybir
    import concourse.tile as tile

    f32 = mybir.dt.float32
    f32r = mybir.dt.float32r
    f8 = mybir.dt.float8e4
    AF = mybir.ActivationFunctionType
    OP = mybir.AluOpType
    DR = mybir.MatmulPerfMode.DoubleRow

    nc = bacc.Bacc()
    x8 = nc.dram_tensor("x8", [CP, 128, 2, N], f8, kind="ExternalInput")
    w8 = {
        nm: nc.dram_tensor(nm, [CP, 128, 2, C], f8, kind="ExternalInput")
        for nm in ("wq8", "wk8", "wpv8")
    }
    ones8_d = nc.dram_tensor("ones8_d", [128, 2, 128], f8,
                             kind="ExternalInput")
    xf = nc.dram_tensor("xf", [C, NQ], f32, kind="ExternalInput")
    vecs = {
        nm: nc.dram_tensor(nm, [C], f32, kind="ExternalInput")
        for nm in ("bq", "bpe", "gn_w", "gn_b")
    }
    gmat_d = nc.dram_tensor("gmat_d", [128, GPT], f32, kind="ExternalInput")
    gmatT_d = nc.dram_tensor("gmatT_d", [GPT, 128], f32, kind="ExternalInput")
    out = nc.dram_tensor("out", [C, NQ], f32, kind="ExternalOutput")

    with tile.TileContext(nc) as tc, ExitStack() as ctx:
        consts = ctx.enter_context(tc.tile_pool(name="consts", bufs=1))

        # ---- resident fp8 inputs ----
        x8t = [consts.tile([128, 2, N], f8, tag=f"x8_{pr}", name=f"x8_{pr}")
               for pr in range(CP)]
        w8t = {nm: [consts.tile([128, 2, C], f8, tag=f"{nm}{pr}",
                                name=f"{nm}{pr}") for pr in range(CP)]
               for nm in ("wq8", "wk8", "wpv8")}
        ones8 = consts.tile([128, 2, 128], f8, tag="ones8", name="ones8")
        nc.scalar.dma_start(out=ones8, in_=ones8_d[:, :, :])

        warm_cm = tc.tile_pool(name="pp_warm", bufs=1, space="PSUM")
        pp_warm = warm_cm.__enter__()
        warm_ps = pp_warm.tile([128, 512], f32, tag="warm", name="warm")
        for ch in range(NC8):
            ns = slice(ch * 512, (ch + 1) * 512)
            for pr in range(CP):
                eng = nc.sync if (ch * CP + pr) % 2 else nc.gpsimd
                eng.dma_start(out=x8t[pr][:, :, ns], in_=x8[pr, :, :, ns])
            # junk matmul per arrival: keeps the PE HAM busy-window alive
            nc.tensor.matmul(
                out=warm_ps, lhsT=x8t[0][:, :, ch * 512:ch * 512 + 128],
                rhs=x8t[0][:, :, ns], start=True, stop=True, perf_mode=DR,
                skip_group_check=True)
        for nm in ("wk8", "wpv8", "wq8"):
            for pr in range(CP):
                eng = nc.sync if pr % 2 else nc.gpsimd
                eng.dma_start(out=w8t[nm][pr], in_=w8[nm][pr, :, :, :])
        # residual (f32 query half), 32KB/part; +bpe folded in below
        xft = [consts.tile([128, NQ], f32, tag=f"xf{ci}", name=f"xf{ci}")
               for ci in range(CT)]
        for ci in range(CT):
            nc.scalar.dma_start(out=xft[ci],
                                in_=xf[ci * 128:(ci + 1) * 128, :])

        # ---- constants ----
        vt = {}
        for nm in ("bq", "bpe", "gn_w", "gn_b"):
            vt[nm] = consts.tile([128, CT], f32, tag=nm, name=nm)
            nc.scalar.dma_start(
                out=vt[nm], in_=vecs[nm].rearrange("(c p) -> p c", p=128))
        eps_t = consts.tile([128, 1], f32, tag="eps", name="eps")
        nc.vector.memset(eps_t, EPS)
        esh_t = consts.tile([128, 1], f32, tag="esh", name="esh")
        nc.vector.memset(esh_t, ESHIFT)
        zero_t = consts.tile([128, 1], f32, tag="zero", name="zero")
        nc.vector.memset(zero_t, 0.0)
        with tc.tile_pool(name="cstage", bufs=2) as cstage:
            gm_st = cstage.tile([128, GPT], f32, tag="c1", name="gm_st")
            nc.sync.dma_start(out=gm_st, in_=gmat_d[:, :])
            gmat = consts.tile([128, GPT], f32r, tag="gmat", name="gmat")
            nc.vector.tensor_copy(out=gmat, in_=gm_st)
            gmT_st = cstage.tile([GPT, 128], f32, tag="c2", name="gmT_st")
            nc.sync.dma_start(out=gmT_st, in_=gmatT_d[:, :])
            gmatT = consts.tile([GPT, 128], f32r, tag="gmatT", name="gmatT")
            nc.vector.tensor_copy(out=gmatT, in_=gmT_st)

        # ---- GroupNorm stats on fp8 X (chunk-wise, as DMAs land) ----
        sc_all = consts.tile([128, CT], f32, tag="sc_all", name="sc_all")
        bi_all = consts.tile([128, CT], f32, tag="bi_all", name="bi_all")
        with tc.tile_pool(name="gn_stats", bufs=1) as gstats, \
             tc.tile_pool(name="pp_gn", bufs=2, space="PSUM") as pp_gn:
            stats = [gstats.tile([128, NC8, 6], f32, tag=f"bnst{ci}",
                                 name=f"bnst{ci}") for ci in range(CT)]
            for ch in range(NC8):
                ns = slice(ch * 512, (ch + 1) * 512)
                for ci in range(CT):
                    nc.vector.bn_stats(out=stats[ci][:, ch, :],
                                       in_=x8t[ci // 2][:, ci % 2, ns])
            rowst_all = gstats.tile([128, CT, 2], f32r, tag="rowst",
                                    name="rowst")
            for ci in range(CT):
                mv = gstats.tile([128, 2], f32, tag=f"mv{ci}", name=f"mv{ci}")
                nc.vector.bn_aggr(out=mv, in_=stats[ci])
                nc.vector.tensor_copy(out=rowst_all[:, ci, 0:1], in_=mv[:, 0:1])
                m2 = gstats.tile([128, 1], f32, tag=f"m2{ci}", name=f"m2{ci}")
                nc.vector.tensor_mul(out=m2, in0=mv[:, 0:1], in1=mv[:, 0:1])
                nc.vector.tensor_add(out=rowst_all[:, ci, 1:2],
                                     in0=mv[:, 1:2], in1=m2)
            # group-reduce 128 rows -> 8 groups -> broadcast back
            gps = pp_gn.tile([GPT, CT, 2], f32, tag="gps", name="gps")
            nc.tensor.matmul(out=gps, lhsT=gmat,
                             rhs=rowst_all.rearrange("p c two -> p (c two)"),
                             start=True, stop=True)
            gsb = gstats.tile([GPT, CT * 2], f32r, tag="gsb", name="gsb")
            nc.vector.tensor_copy(out=gsb,
                                  in_=gps.rearrange("g c two -> g (c two)"))
            bps = pp_gn.tile([128, CT, 2], f32, tag="bps", name="bps")
            nc.tensor.matmul(out=bps, lhsT=gmatT, rhs=gsb,
                             start=True, stop=True)
            gstat = gstats.tile([128, CT, 2], f32, tag="gstat", name="gstat")
            nc.scalar.mul(out=gstat, in_=bps, mul=1.0 / GSZ)
            means = gstat[:, :, 0:1].rearrange("p c one -> p (c one)")
            m2s = gstat[:, :, 1:2].rearrange("p c one -> p (c one)")
            var = gstats.tile([128, CT], f32, tag="var", name="var")
            mm_ = gstats.tile([128, CT], f32, tag="mm_", name="mm_")
            nc.vector.tensor_mul(out=mm_, in0=means, in1=means)
            nc.vector.tensor_sub(out=var, in0=m2s, in1=mm_)
            nc.scalar.activation(out=var, in_=var, func=AF.Sqrt,
                                 bias=eps_t, scale=1.0)
            rstd = gstats.tile([128, CT], f32, tag="rstd", name="rstd")
            nc.vector.reciprocal(out=rstd, in_=var)
            nc.vector.tensor_mul(out=sc_all, in0=rstd, in1=vt["gn_w"])
            msc = gstats.tile([128, CT], f32, tag="msc", name="msc")
            nc.vector.tensor_mul(out=msc, in0=means, in1=sc_all)
            nc.vector.tensor_sub(out=bi_all, in0=vt["gn_b"], in1=msc)

        # fold proj bias into the residual tiles (off critical path, Pool)
        for ci in range(CT):
            nc.gpsimd.tensor_scalar_add(out=xft[ci], in0=xft[ci],
                                        scalar1=vt["bpe"][:, ci:ci + 1])

        # ---- hn8 + K/VP/Q projections (all DoubleRow fp8) ----
        hn8 = [consts.tile([128, 2, N], f8, tag=f"hn8_{pr}", name=f"hn8_{pr}")
               for pr in range(CP)]
        k8 = [consts.tile([128, 2, N], f8, tag=f"k8_{pr}", name=f"k8_{pr}")
              for pr in range(CP)]
        q8 = [consts.tile([128, 2, NQ], f8, tag=f"q8_{pr}", name=f"q8_{pr}")
              for pr in range(CP)]
        vp8 = [consts.tile([128, 2, C], f8, tag=f"vp8_{p}", name=f"vp8_{p}")
               for p in range(NTP)]

        with tc.tile_pool(name="pp_proj", bufs=6, space="PSUM") as pp_proj:
            for ch in range(NC8):
                ns = slice(ch * 512, (ch + 1) * 512)
                # hn8 on the Pool engine (SBUF->SBUF only there)
                for ci in range(CT):
                    pr, i = ci // 2, ci % 2
                    nc.gpsimd.tensor_scalar(
                        out=hn8[pr][:, i, ns], in0=x8t[pr][:, i, ns],
                        scalar1=sc_all[:, ci:ci + 1],
                        scalar2=bi_all[:, ci:ci + 1],
                        op0=OP.mult, op1=OP.add)
                # K chunk (no bias: cancels in softmax)
                for co in range(CT):
                    ps = pp_proj.tile([128, 512], f32, tag="pps", name="k_ps")
                    for pr in range(CP):
                        nc.tensor.matmul(
                            out=ps,
                            lhsT=w8t["wk8"][pr][:, :, co * 128:(co + 1) * 128],
                            rhs=hn8[pr][:, :, ns],
                            start=(pr == 0), stop=(pr == CP - 1), perf_mode=DR)
                    if co % 2:
                        nc.vector.tensor_copy(out=k8[co // 2][:, co % 2, ns],
                                              in_=ps)
                    else:
                        nc.scalar.activation(out=k8[co // 2][:, co % 2, ns],
                                             in_=ps, func=AF.Identity,
                                             bias=zero_t, scale=1.0)
                # VP chunk: 4 key tiles [k 128, c_out 512] of wpv @ Hn
                for nt4 in range(4):
                    nt = ch * 4 + nt4
                    ps = pp_proj.tile([128, 512], f32, tag="pps", name="v_ps")
                    for pr in range(CP):
                        nc.tensor.matmul(
                            out=ps,
                            lhsT=hn8[pr][:, :, nt * 128:(nt + 1) * 128],
                            rhs=w8t["wpv8"][pr],
                            start=(pr == 0), stop=(pr == CP - 1), perf_mode=DR)
                    if nt4 % 2:
                        nc.vector.tensor_copy(out=vp8[nt // 2][:, nt % 2, :],
                                              in_=ps)
                    else:
                        nc.scalar.activation(out=vp8[nt // 2][:, nt % 2, :],
                                             in_=ps, func=AF.Identity,
                                             bias=zero_t, scale=1.0)
                if ch < QC:  # queries are always chunks 0..3 (host swaps)
                    for co in range(CT):
                        ps = pp_proj.tile([128, 512], f32, tag="pps",
                                          name="q_ps")
                        for pr in range(CP):
                            nc.tensor.matmul(
                                out=ps,
                                lhsT=w8t["wq8"][pr][:, :,
                                                    co * 128:(co + 1) * 128],
                                rhs=hn8[pr][:, :, ns],
                                start=(pr == 0), stop=(pr == CP - 1),
                                perf_mode=DR)
                        if co % 2:
                            nc.vector.tensor_scalar_add(
                                out=q8[co // 2][:, co % 2, ns], in0=ps,
                                scalar1=vt["bq"][:, co:co + 1])
                        else:
                            nc.scalar.activation(
                                out=q8[co // 2][:, co % 2, ns], in_=ps,
                                func=AF.Identity,
                                bias=vt["bq"][:, co:co + 1], scale=1.0)

        warm_cm.__exit__(None, None, None)

        # ---- attention ----
        with tc.tile_pool(name="es_pool", bufs=1) as es_pool, \
             tc.tile_pool(name="work", bufs=2) as work, \
             tc.tile_pool(name="pp_s", bufs=2, space="PSUM") as pp_s, \
             tc.tile_pool(name="pp_acc", bufs=1, space="PSUM") as pp_acc:
            es8 = [es_pool.tile([128, 2, 512], f8, tag=f"es{p}",
                                name=f"es{p}") for p in range(NTP)]
            for qc in range(QC):
                qs = slice(qc * 512, (qc + 1) * 512)
                acc_ps = [pp_acc.tile([128, 512], f32, tag=f"acc{ct}",
                                      name=f"acc{ct}") for ct in range(CT)]
                for ktp in range(NTP):
                    s_ps = pp_s.tile([128, 2, 512], f32, tag="s_ps",
                                     name="s_ps")
                    for i in range(2):
                        kt = 2 * ktp + i
                        for pr in range(CP):
                            nc.tensor.matmul(
                                out=s_ps[:, i, :],
                                lhsT=k8[pr][:, :, kt * 128:(kt + 1) * 128],
                                rhs=q8[pr][:, :, qs],
                                start=(pr == 0), stop=(pr == CP - 1),
                                perf_mode=DR)
                    nc.scalar.activation(out=es8[ktp], in_=s_ps, func=AF.Exp,
                                         scale=SCALE, bias=esh_t)
                    for ct in range(CT):
                        nc.tensor.matmul(
                            out=acc_ps[ct],
                            lhsT=vp8[ktp][:, :, ct * 128:(ct + 1) * 128],
                            rhs=es8[ktp],
                            start=(ktp == 0), stop=(ktp == NTP - 1),
                            perf_mode=DR)

                # sums post-pass into a recycled S slot (all rows identical)
                sums_ps = pp_s.tile([128, 2, 512], f32, tag="s_ps",
                                    name="sums")
                for ktp in range(NTP):
                    nc.tensor.matmul(
                        out=sums_ps[:, 0, :], lhsT=ones8, rhs=es8[ktp],
                        start=(ktp == 0), stop=(ktp == NTP - 1), perf_mode=DR)
                inv = work.tile([128, 512], f32, tag="inv", name="inv")
                nc.vector.reciprocal(out=inv, in_=sums_ps[:, 0, :])
                for ct in range(CT):
                    ot = work.tile([128, 512], f32, tag="ot", name="ot",
                                   bufs=3)
                    nc.vector.tensor_mul(out=ot, in0=acc_ps[ct], in1=inv)
                    nc.gpsimd.tensor_add(out=ot, in0=ot, in1=xft[ct][:, qs])
                    nc.sync.dma_start(out=out[ct * 128:(ct + 1) * 128, qs],
                                      in_=ot)

    nc.compile()
    return nc


def _get_nc():
    if "nc" not in _CACHE:
        _CACHE["nc"] = _build()
    return _CACHE["nc"]


def _pair8(a):
    """[C, F] f32 -> fp8 DoubleRow pair layout [CP, 128, 2, F]."""
    a8 = np.clip(a, -240.0, 240.0).astype(F8NP)
    return np.ascontiguousarray(
        a8.reshape(CP, 2, 128, a.shape[1]).transpose(0, 2, 1, 3))


def _prep_in_maps(X, gn_w, gn_b, wq, bq, wk, bk, wv, bv, wp, bp):
    f = lambda a: np.ascontiguousarray(np.asarray(a, dtype=np.float32))
    X = f(X)
    gn_w, gn_b, bq, bk, bv, bp = map(f, (gn_w, gn_b, bq, bk, bv, bp))
    wq, wk, wv, wp = map(f, (wq, wk, wv, wp))

    Xf = X.reshape(B, C, N)
    bpe = wp @ bv + bp  # bv folded through proj_out (softmax rows sum to 1)
    wpv = (wp.astype(np.float64) @ wv.astype(np.float64)).astype(np.float32)
    w8 = {"wq8": _pair8(np.ascontiguousarray(wq.T)),
          "wk8": _pair8(np.ascontiguousarray(wk.T)),
          "wpv8": _pair8(np.ascontiguousarray(wpv.T))}

    gmat = np.zeros((128, GPT), np.float32)
    for g in range(GPT):
        gmat[g * GSZ:(g + 1) * GSZ, g] = 1.0
    gmatT = np.ascontiguousarray(gmat.T)
    ones8 = np.ones((128, 2, 128), F8NP)

    in_maps = []
    for core in range(8):
        bi, half = core // 2, core % 2
        x8p = _pair8(Xf[bi])
        if half:
            # swap key halves so queries are always columns 0..NQ
            x8p = np.ascontiguousarray(
                np.concatenate((x8p[..., NQ:], x8p[..., :NQ]), axis=-1))
        in_maps.append({
            "x8": x8p,
            "xf": np.ascontiguousarray(Xf[bi][:, half * NQ:(half + 1) * NQ]),
            **w8,
            "ones8_d": ones8,
            "bq": bq, "bpe": bpe, "gn_w": gn_w, "gn_b": gn_b,
            "gmat_d": gmat, "gmatT_d": gmatT,
        })
    return in_maps


_last_in_maps = None


def kernel(X, gn_w, gn_b, wq, bq, wk, bk, wv, bv, wp, bp):
    from concourse.bass_utils import run_bass_kernel_spmd

    global _last_in_maps
    in_maps = _prep_in_maps(X, gn_w, gn_b, wq, bq, wk, bk, wv, bv, wp, bp)
    _last_in_maps = in_maps
    nc = _get_nc()
    res = run_bass_kernel_spmd(nc, in_maps, list(range(8)))
    out = np.empty((B, C, N), np.float32)
    for core in range(8):
        bi, half = core // 2, core % 2
        out[bi][:, half * NQ:(half + 1) * NQ] = res.results[core]["out"]
    return out.reshape(B, C, H, W)


# revision 14
# speedup vs baseline: 2.2039x; 1.3952x over previous
"""AttnBlock (GroupNorm + single-head self-attention + residual) on 8 trn2 cores.

Problem: X [4, 512, 64, 64] f32. Per batch element: GroupNorm(32 groups), then
1x1-conv Q/K/V projections, softmax attention over n=h*w=4096 positions,
proj_out, residual add.  8 cores = 4 batch elements x 2 query-halves.

v2 strategy: fp8e4m3 DoubleRow matmuls (256-deep contraction per instruction,
2 rows/PE-cycle) for every large matmul, plus algebraic fusions that shrink
the graph:

  - proj_out is folded into the V projection on the HOST: wpv = wp @ wv, so
    the attention accumulator directly produces the (unnormalized) projected
    output. No HoT materialization, no separate P-proj matmuls, one less fp8
    rounding stage.
  - K's bias adds a per-query constant to the logits -> cancels in softmax.
    Dropped. V's bias is routed through proj_out (pbe = wp @ bv + bp, added
    to the residual tiles once). Softmax itself is unnormalized
    exp(S*scale - 3.5); the shift cancels in the final normalization and
    keeps es inside fp8e4's +-240 range with no max pass / NaN risk.
  - Host pre-quantizes X and weights to fp8e4m3 in DoubleRow pair layout
    [pair, part, 2, free]. For the half=1 core of each pair the two key
    halves of x8 are swapped so the kernel always treats columns 0..2047 as
    its queries (softmax is permutation-invariant over keys).
  - GroupNorm stats run on the fp8 X (bn_stats per chunk as DMAs land);
    sc/bi fold into the fp8 activation hn8 via one Pool-engine
    tensor_scalar pass (Pool has no PSUM port, so it gets all SBUF-only
    work; every PSUM->SBUF move runs on DVE or ACT).
  - Attention inner loop per key-tile pair: 4 DR matmuls for S^T[k,q] into
    a 2-bank PSUM tile, one ACT exp (psum->fp8 SBUF), 4 DR matmuls
    accumulating out_un[c,q] (4 banks), with es8 kept resident; row sums
    run as a post-pass of ones-lhsT DR matmuls into a recycled S-pool slot
    (all 128 output partitions identical -> no partition broadcast).
  - Tail per query chunk: reciprocal, out = out_un * inv + (X + pbe), DMA.
  - Junk DR matmuls tied to x8 DMA arrivals keep the PE HAM clock-gate warm
    through the prologue.

PSUM in attention: 2x2-bank S tiles + 4 accumulator banks = 8 exactly.
"""

import numpy as np
import ml_dtypes

B, C, H, W = 4, 512, 64, 64
N = H * W            # 4096 keys per batch element
NQ = N // 2          # 2048 queries per core
CT = C // 128        # 4 channel tiles
CP = CT // 2         # 2 channel-tile pairs (DoubleRow)
NT = N // 128        # 32 key tiles
NTP = NT // 2        # 16 key-tile pairs
QC = NQ // 512       # 4 query chunks of 512
NC8 = N // 512       # 8 key chunks of 512
GROUPS = 32
GPT = GROUPS // CT   # 8 groups per 128-channel tile
GSZ = C // GROUPS    # 16 channels per group
EPS = 1e-5
SCALE = float(C) ** -0.5
ESHIFT = -3.5

_CACHE = {}
F8NP = ml_dtypes.float8_e4m3


def _build():
    from contextlib import ExitStack
    from concourse import bacc
    import concourse.mybir as mybir
    import concourse.tile as tile

    f32 = mybir.dt.float32
    f32r = mybir.dt.float32r
    f8 = mybir.dt.float8e4
    AF = mybir.ActivationFunctionType
    OP = mybir.AluOpType
    DR = mybir.MatmulPerfMode.DoubleRow

    nc = bacc.Bacc()
    x8 = nc.dram_tensor("x8", [CP, 128, 2, N], f8, kind="ExternalInput")
    w8 = {
        nm: nc.dram_tensor(nm, [CP, 128, 2, C], f8, kind="ExternalInput")
        for nm in ("wq8", "wk8", "wpv8")
    }
    ones8_d = nc.dram_tensor("ones8_d", [128, 2, 128], f8,
                             kind="ExternalInput")
    xf = nc.dram_tensor("xf", [C, NQ], f32, kind="ExternalInput")
    vecs = {
        nm: nc.dram_tensor(nm, [C], f32, kind="ExternalInput")
        for nm in ("bq", "gn_w", "gn_b")
    }
    gmat_d = nc.dram_tensor("gmat_d", [128, GPT], f32, kind="ExternalInput")
    gmatT_d = nc.dram_tensor("gmatT_d", [GPT, 128], f32, kind="ExternalInput")
    out = nc.dram_tensor("out", [C, NQ], f32, kind="ExternalOutput")

    with tile.TileContext(nc) as tc, ExitStack() as ctx:
        consts = ctx.enter_context(tc.tile_pool(name="consts", bufs=1))

        # ---- resident fp8 inputs ----
        x8t = [consts.tile([128, 2, N], f8, tag=f"x8_{pr}", name=f"x8_{pr}")
               for pr in range(CP)]
        w8t = {nm: [consts.tile([128, 2, C], f8, tag=f"{nm}{pr}",
                                name=f"{nm}{pr}") for pr in range(CP)]
               for nm in ("wq8", "wk8", "wpv8")}
        ones8 = consts.tile([128, 2, 128], f8, tag="ones8", name="ones8")
        nc.scalar.dma_start(out=ones8, in_=ones8_d[:, :, :])

        warm_cm = tc.tile_pool(name="pp_warm", bufs=1, space="PSUM")
        pp_warm = warm_cm.__enter__()
        warm_ps = pp_warm.tile([128, 512], f32, tag="warm", name="warm")
        # dense burst first: the HAM clock-gate opens only after ~3.4us of
        # SUSTAINED PE activity; isolated blips never reach 2.4 GHz
        junk8 = consts.tile([128, 2, 512], f8, tag="junk8", name="junk8")
        nc.vector.memset(junk8, 0.25)

        def junk_mm(n):
            for _ in range(n):
                nc.tensor.matmul(
                    out=warm_ps, lhsT=junk8[:, :, :128], rhs=junk8,
                    start=True, stop=True, perf_mode=DR,
                    skip_group_check=True)

        junk_mm(40)
        dma_engs = (nc.sync, nc.gpsimd, nc.scalar)
        for ch in range(NC8):
            ns = slice(ch * 512, (ch + 1) * 512)
            for pr in range(CP):
                eng = dma_engs[(ch * CP + pr) % 3]
                eng.dma_start(out=x8t[pr][:, :, ns], in_=x8[pr, :, :, ns])
            # blip per arrival: keeps the PE HAM busy-window alive
            nc.tensor.matmul(
                out=warm_ps, lhsT=x8t[0][:, :, ch * 512:ch * 512 + 128],
                rhs=x8t[0][:, :, ns], start=True, stop=True, perf_mode=DR,
                skip_group_check=True)
        for nm in ("wk8", "wpv8", "wq8"):
            for pr in range(CP):
                eng = nc.sync if pr % 2 else nc.gpsimd
                eng.dma_start(out=w8t[nm][pr], in_=w8[nm][pr, :, :, :])
        # residual (f32 query half), 32KB/part; +bpe folded in below
        xft = [consts.tile([128, NQ], f32, tag=f"xf{ci}", name=f"xf{ci}")
               for ci in range(CT)]
        for ci in range(CT):
            nc.scalar.dma_start(out=xft[ci],
                                in_=xf[ci * 128:(ci + 1) * 128, :])

        # ---- constants ----
        vt = {}
        for nm in ("bq", "gn_w", "gn_b"):
            vt[nm] = consts.tile([128, CT], f32, tag=nm, name=nm)
            nc.scalar.dma_start(
                out=vt[nm], in_=vecs[nm].rearrange("(c p) -> p c", p=128))
        eps_t = consts.tile([128, 1], f32, tag="eps", name="eps")
        nc.vector.memset(eps_t, EPS)
        esh_t = consts.tile([128, 1], f32, tag="esh", name="esh")
        nc.vector.memset(esh_t, ESHIFT)
        zero_t = consts.tile([128, 1], f32, tag="zero", name="zero")
        nc.vector.memset(zero_t, 0.0)
        with tc.tile_pool(name="cstage", bufs=2) as cstage:
            gm_st = cstage.tile([128, GPT], f32, tag="c1", name="gm_st")
            nc.sync.dma_start(out=gm_st, in_=gmat_d[:, :])
            gmat = consts.tile([128, GPT], f32r, tag="gmat", name="gmat")
            nc.vector.tensor_copy(out=gmat, in_=gm_st)
            gmT_st = cstage.tile([GPT, 128], f32, tag="c2", name="gmT_st")
            nc.sync.dma_start(out=gmT_st, in_=gmatT_d[:, :])
            gmatT = consts.tile([GPT, 128], f32r, tag="gmatT", name="gmatT")
            nc.vector.tensor_copy(out=gmatT, in_=gmT_st)

        # ---- GroupNorm stats on fp8 X (chunk-wise, as DMAs land) ----
        sc_all = consts.tile([128, CT], f32, tag="sc_all", name="sc_all")
        bi_all = consts.tile([128, CT], f32, tag="bi_all", name="bi_all")
        with tc.tile_pool(name="gn_stats", bufs=1) as gstats, \
             tc.tile_pool(name="pp_gn", bufs=2, space="PSUM") as pp_gn:
            stats = [gstats.tile([128, NC8, 6], f32, tag=f"bnst{ci}",
                                 name=f"bnst{ci}") for ci in range(CT)]
            for ch in range(NC8):
                ns = slice(ch * 512, (ch + 1) * 512)
                for ci in range(CT):
                    nc.vector.bn_stats(out=stats[ci][:, ch, :],
                                       in_=x8t[ci // 2][:, ci % 2, ns])
            rowst_all = gstats.tile([128, CT, 2], f32r, tag="rowst",
                                    name="rowst")
            for ci in range(CT):
                mv = gstats.tile([128, 2], f32, tag=f"mv{ci}", name=f"mv{ci}")
                nc.vector.bn_aggr(out=mv, in_=stats[ci])
                nc.vector.tensor_copy(out=rowst_all[:, ci, 0:1], in_=mv[:, 0:1])
                m2 = gstats.tile([128, 1], f32, tag=f"m2{ci}", name=f"m2{ci}")
                nc.vector.tensor_mul(out=m2, in0=mv[:, 0:1], in1=mv[:, 0:1])
                nc.vector.tensor_add(out=rowst_all[:, ci, 1:2],
                                     in0=mv[:, 1:2], in1=m2)
            # group-reduce 128 rows -> 8 groups -> broadcast back
            gps = pp_gn.tile([GPT, CT, 2], f32, tag="gps", name="gps")
            nc.tensor.matmul(out=gps, lhsT=gmat,
                             rhs=rowst_all.rearrange("p c two -> p (c two)"),
                             start=True, stop=True)
            gsb = gstats.tile([GPT, CT * 2], f32r, tag="gsb", name="gsb")
            nc.vector.tensor_copy(out=gsb,
                                  in_=gps.rearrange("g c two -> g (c two)"))
            bps = pp_gn.tile([128, CT, 2], f32, tag="bps", name="bps")
            nc.tensor.matmul(out=bps, lhsT=gmatT, rhs=gsb,
                             start=True, stop=True)
            junk_mm(12)  # keep the PE warm through the sc/bi vector chain
            gstat = gstats.tile([128, CT, 2], f32, tag="gstat", name="gstat")
            nc.scalar.mul(out=gstat, in_=bps, mul=1.0 / GSZ)
            means = gstat[:, :, 0:1].rearrange("p c one -> p (c one)")
            m2s = gstat[:, :, 1:2].rearrange("p c one -> p (c one)")
            var = gstats.tile([128, CT], f32, tag="var", name="var")
            mm_ = gstats.tile([128, CT], f32, tag="mm_", name="mm_")
            nc.vector.tensor_mul(out=mm_, in0=means, in1=means)
            nc.vector.tensor_sub(out=var, in0=m2s, in1=mm_)
            nc.scalar.activation(out=var, in_=var, func=AF.Sqrt,
                                 bias=eps_t, scale=1.0)
            rstd = gstats.tile([128, CT], f32, tag="rstd", name="rstd")
            nc.vector.reciprocal(out=rstd, in_=var)
            nc.vector.tensor_mul(out=sc_all, in0=rstd, in1=vt["gn_w"])
            msc = gstats.tile([128, CT], f32, tag="msc", name="msc")
            nc.vector.tensor_mul(out=msc, in0=means, in1=sc_all)
            nc.vector.tensor_sub(out=bi_all, in0=vt["gn_b"], in1=msc)

        # ---- hn8 + K/VP/Q projections (all DoubleRow fp8) ----
        hn8 = [consts.tile([128, 2, N], f8, tag=f"hn8_{pr}", name=f"hn8_{pr}")
               for pr in range(CP)]
        k8 = [consts.tile([128, 2, N], f8, tag=f"k8_{pr}", name=f"k8_{pr}")
              for pr in range(CP)]
        q8 = [consts.tile([128, 2, NQ], f8, tag=f"q8_{pr}", name=f"q8_{pr}")
              for pr in range(CP)]
        vp8 = [consts.tile([128, 2, C], f8, tag=f"vp8_{p}", name=f"vp8_{p}")
               for p in range(NTP)]

        with tc.tile_pool(name="pp_proj", bufs=6, space="PSUM") as pp_proj:
            for ch in range(NC8):
                ns = slice(ch * 512, (ch + 1) * 512)
                # hn8 on the Pool engine (SBUF->SBUF only there)
                for ci in range(CT):
                    pr, i = ci // 2, ci % 2
                    nc.gpsimd.tensor_scalar(
                        out=hn8[pr][:, i, ns], in0=x8t[pr][:, i, ns],
                        scalar1=sc_all[:, ci:ci + 1],
                        scalar2=bi_all[:, ci:ci + 1],
                        op0=OP.mult, op1=OP.add)
                # K chunk (no bias: cancels in softmax)
                for co in range(CT):
                    ps = pp_proj.tile([128, 512], f32, tag="pps", name="k_ps")
                    for pr in range(CP):
                        nc.tensor.matmul(
                            out=ps,
                            lhsT=w8t["wk8"][pr][:, :, co * 128:(co + 1) * 128],
                            rhs=hn8[pr][:, :, ns],
                            start=(pr == 0), stop=(pr == CP - 1), perf_mode=DR)
                    if co % 2:
                        nc.vector.tensor_copy(out=k8[co // 2][:, co % 2, ns],
                                              in_=ps)
                    else:
                        nc.scalar.activation(out=k8[co // 2][:, co % 2, ns],
                                             in_=ps, func=AF.Identity,
                                             bias=zero_t, scale=1.0)
                # VP chunk: 4 key tiles [k 128, c_out 512] of wpv @ Hn
                for nt4 in range(4):
                    nt = ch * 4 + nt4
                    ps = pp_proj.tile([128, 512], f32, tag="pps", name="v_ps")
                    for pr in range(CP):
                        nc.tensor.matmul(
                            out=ps,
                            lhsT=hn8[pr][:, :, nt * 128:(nt + 1) * 128],
                            rhs=w8t["wpv8"][pr],
                            start=(pr == 0), stop=(pr == CP - 1), perf_mode=DR)
                    if nt4 % 2:
                        nc.vector.tensor_copy(out=vp8[nt // 2][:, nt % 2, :],
                                              in_=ps)
                    else:
                        nc.scalar.activation(out=vp8[nt // 2][:, nt % 2, :],
                                             in_=ps, func=AF.Identity,
                                             bias=zero_t, scale=1.0)
                if ch < QC:  # queries are always chunks 0..3 (host swaps)
                    for co in range(CT):
                        ps = pp_proj.tile([128, 512], f32, tag="pps",
                                          name="q_ps")
                        for pr in range(CP):
                            nc.tensor.matmul(
                                out=ps,
                                lhsT=w8t["wq8"][pr][:, :,
                                                    co * 128:(co + 1) * 128],
                                rhs=hn8[pr][:, :, ns],
                                start=(pr == 0), stop=(pr == CP - 1),
                                perf_mode=DR)
                        if co % 2:
                            nc.vector.tensor_scalar_add(
                                out=q8[co // 2][:, co % 2, ns], in0=ps,
                                scalar1=vt["bq"][:, co:co + 1])
                        else:
                            nc.scalar.activation(
                                out=q8[co // 2][:, co % 2, ns], in_=ps,
                                func=AF.Identity,
                                bias=vt["bq"][:, co:co + 1], scale=1.0)

        warm_cm.__exit__(None, None, None)

        # ---- attention ----
        with tc.tile_pool(name="es_pool", bufs=1) as es_pool, \
             tc.tile_pool(name="work", bufs=2) as work, \
             tc.tile_pool(name="pp_s", bufs=2, space="PSUM") as pp_s, \
             tc.tile_pool(name="pp_acc", bufs=1, space="PSUM") as pp_acc:
            es8 = [es_pool.tile([128, 2, 512], f8, tag=f"es{p}",
                                name=f"es{p}") for p in range(NTP)]
            for qc in range(QC):
                qs = slice(qc * 512, (qc + 1) * 512)
                acc_ps = [pp_acc.tile([128, 512], f32, tag=f"acc{ct}",
                                      name=f"acc{ct}") for ct in range(CT)]
                for ktp in range(NTP):
                    s_ps = pp_s.tile([128, 2, 512], f32, tag="s_ps",
                                     name="s_ps")
                    for i in range(2):
                        kt = 2 * ktp + i
                        for pr in range(CP):
                            nc.tensor.matmul(
                                out=s_ps[:, i, :],
                                lhsT=k8[pr][:, :, kt * 128:(kt + 1) * 128],
                                rhs=q8[pr][:, :, qs],
                                start=(pr == 0), stop=(pr == CP - 1),
                                perf_mode=DR)
                    nc.scalar.activation(out=es8[ktp], in_=s_ps, func=AF.Exp,
                                         scale=SCALE, bias=esh_t)
                    for ct in range(CT):
                        nc.tensor.matmul(
                            out=acc_ps[ct],
                            lhsT=vp8[ktp][:, :, ct * 128:(ct + 1) * 128],
                            rhs=es8[ktp],
                            start=(ktp == 0), stop=(ktp == NTP - 1),
                            perf_mode=DR)

                # sums post-pass into a recycled S slot (all rows identical)
                sums_ps = pp_s.tile([128, 2, 512], f32, tag="s_ps",
                                    name="sums")
                for ktp in range(NTP):
                    nc.tensor.matmul(
                        out=sums_ps[:, 0, :], lhsT=ones8, rhs=es8[ktp],
                        start=(ktp == 0), stop=(ktp == NTP - 1), perf_mode=DR)
                # inv = exp(-ln(sums)) on ACT: DVE reciprocal is ~3.4us and
                # holds the psum slot; Ln releases it in ~0.7us
                lns = work.tile([128, 512], f32, tag="lns", name="lns")
                nc.scalar.activation(out=lns, in_=sums_ps[:, 0, :],
                                     func=AF.Ln, bias=zero_t, scale=1.0)
                inv = work.tile([128, 512], f32, tag="inv", name="inv")
                nc.scalar.activation(out=inv, in_=lns, func=AF.Exp,
                                     bias=zero_t, scale=-1.0)
                for ct in range(CT):
                    ot = work.tile([128, 512], f32, tag="ot", name="ot",
                                   bufs=3)
                    nc.vector.tensor_mul(out=ot, in0=acc_ps[ct], in1=inv)
                    ot2 = work.tile([128, 512], f32, tag="ot2", name="ot2",
                                    bufs=3)
                    nc.gpsimd.tensor_add(out=ot2, in0=ot, in1=xft[ct][:, qs])
                    nc.sync.dma_start(out=out[ct * 128:(ct + 1) * 128, qs],
                                      in_=ot2)

    nc.compile()
    return nc


def _get_nc():
    if "nc" not in _CACHE:
        _CACHE["nc"] = _build()
    return _CACHE["nc"]


def _pair8(a):
    """[C, F] f32 -> fp8 DoubleRow pair layout [CP, 128, 2, F]."""
    a8 = np.clip(a, -240.0, 240.0).astype(F8NP)
    return np.ascontiguousarray(
        a8.reshape(CP, 2, 128, a.shape[1]).transpose(0, 2, 1, 3))


def _prep_in_maps(X, gn_w, gn_b, wq, bq, wk, bk, wv, bv, wp, bp):
    f = lambda a: np.ascontiguousarray(np.asarray(a, dtype=np.float32))
    X = f(X)
    gn_w, gn_b, bq, bk, bv, bp = map(f, (gn_w, gn_b, bq, bk, bv, bp))
    wq, wk, wv, wp = map(f, (wq, wk, wv, wp))

    Xf = X.reshape(B, C, N)
    bpe = wp @ bv + bp  # bv folded through proj_out (softmax rows sum to 1)
    # bpe is folded into the residual input on the host: xf = X_half + bpe
    wpv = (wp.astype(np.float64) @ wv.astype(np.float64)).astype(np.float32)
    w8 = {"wq8": _pair8(np.ascontiguousarray(wq.T)),
          "wk8": _pair8(np.ascontiguousarray(wk.T)),
          "wpv8": _pair8(np.ascontiguousarray(wpv.T))}

    gmat = np.zeros((128, GPT), np.float32)
    for g in range(GPT):
        gmat[g * GSZ:(g + 1) * GSZ, g] = 1.0
    gmatT = np.ascontiguousarray(gmat.T)
    ones8 = np.ones((128, 2, 128), F8NP)

    in_maps = []
    for core in range(8):
        bi, half = core // 2, core % 2
        x8p = _pair8(Xf[bi])
        if half:
            # swap key halves so queries are always columns 0..NQ
            x8p = np.ascontiguousarray(
                np.concatenate((x8p[..., NQ:], x8p[..., :NQ]), axis=-1))
        in_maps.append({
            "x8": x8p,
            "xf": np.ascontiguousarray(
                Xf[bi][:, half * NQ:(half + 1) * NQ] + bpe[:, None]),
            **w8,
            "ones8_d": ones8,
            "bq": bq, "gn_w": gn_w, "gn_b": gn_b,
            "gmat_d": gmat, "gmatT_d": gmatT,
        })
    return in_maps


_last_in_maps = None


def kernel(X, gn_w, gn_b, wq, bq, wk, bk, wv, bv, wp, bp):
    from concourse.bass_utils import run_bass_kernel_spmd

    global _last_in_maps
    in_maps = _prep_in_maps(X, gn_w, gn_b, wq, bq, wk, bk, wv, bv, wp, bp)
    _last_in_maps = in_maps
    nc = _get_nc()
    res = run_bass_kernel_spmd(nc, in_maps, list(range(8)))
    out = np.empty((B, C, N), np.float32)
    for core in range(8):
        bi, half = core // 2, core % 2
        out[bi][:, half * NQ:(half + 1) * NQ] = res.results[core]["out"]
    return out.reshape(B, C, H, W)


# revision 19
# speedup vs baseline: 2.4614x; 1.1168x over previous
"""AttnBlock (GroupNorm + single-head self-attention + residual) on 8 trn2 cores.

Problem: X [4, 512, 64, 64] f32. Per batch element: GroupNorm(32 groups), then
1x1-conv Q/K/V projections, softmax attention over n=h*w=4096 positions,
proj_out, residual add.  8 cores = 4 batch elements x 2 query-halves.

v4 strategy: fp8e4m3 DoubleRow matmuls (256-deep contraction per instruction;
measured 213ns per [128x512] matmul = the fp8 roofline) plus algebraic
fusions that shrink the graph:

  - A-matrix trick: S = Hn^T (wk^T wq) Hn. The host computes A = wk^T @ wq,
    the kernel projects T = A^T @ Hn once, and S-tiles contract T against
    raw hn8 -- the separate Q projection (and its PSUM moves) disappears.
    Valid when bq == 0 (true here; a general Q-path variant is kept for
    nonzero bq). K's bias shifts every logit of a query equally -> cancels
    in softmax -> dropped always.
  - proj_out folded into the V projection on the host (wpv = wp @ wv): the
    attention accumulator directly produces the projected output; V's bias
    rides the residual (host adds pbe = wp @ bv + bp into xf).
  - Unnormalized softmax exp(S*scale - 3.5): shift cancels in the final
    normalization, keeps es inside fp8e4 range, no max pass, no NaN risk.
    inv = exp(-ln(sums)) on ACT (DVE reciprocal is 3.4us and holds a PSUM
    slot; Ln frees it in 0.7us).
  - GroupNorm stats on the fp8 X: bn_stats in 1024-column batches as DMAs
    land (keeps DVE ahead of the stream); group reduce via two tiny PE
    matmuls; hn8 = sc*x8+bi in one Pool-engine tensor_scalar pass (Pool has
    no PSUM port, so it gets exactly the SBUF-only work).
  - HAM clock-gate management: a dense 40-matmul junk burst at t=0 opens
    the 2.4GHz gate (isolated blips never do); junk blips tied to each
    stats batch + DMA arrival keep it open through the prologue.
  - Host pre-quantizes X/weights to fp8 DoubleRow pair layout
    [pair, part, 2, free]; for half=1 cores the key halves of x8 are
    swapped so queries are always columns 0..2047 (softmax is permutation
    invariant over keys).

PSUM in attention: 2x2-bank S tiles + 4 accumulator banks = 8 exactly; the
row-sums pass recycles an S slot after the kt loop.
"""

import numpy as np
import ml_dtypes

B, C, H, W = 4, 512, 64, 64
N = H * W            # 4096 keys per batch element
NQ = N // 2          # 2048 queries per core
CT = C // 128        # 4 channel tiles
CP = CT // 2         # 2 channel-tile pairs (DoubleRow)
NT = N // 128        # 32 key tiles
NTP = NT // 2        # 16 key-tile pairs
QC = NQ // 512       # 4 query chunks of 512
NC8 = N // 512       # 8 key chunks of 512
GROUPS = 32
GPT = GROUPS // CT   # 8 groups per 128-channel tile
GSZ = C // GROUPS    # 16 channels per group
EPS = 1e-5
SCALE = float(C) ** -0.5
ESHIFT = -3.5

_CACHE = {}
F8NP = ml_dtypes.float8_e4m3


def _build(qfold=True):
    from contextlib import ExitStack
    from concourse import bacc
    import concourse.mybir as mybir
    import concourse.tile as tile

    f32 = mybir.dt.float32
    f32r = mybir.dt.float32r
    f8 = mybir.dt.float8e4
    AF = mybir.ActivationFunctionType
    OP = mybir.AluOpType
    DR = mybir.MatmulPerfMode.DoubleRow

    nc = bacc.Bacc()
    x8 = nc.dram_tensor("x8", [CP, 128, 2, N], f8, kind="ExternalInput")
    wnames = ("a8", "wpv8") if qfold else ("a8", "wpv8", "wq8")
    w8 = {nm: nc.dram_tensor(nm, [CP, 128, 2, C], f8, kind="ExternalInput")
          for nm in wnames}
    ones8_d = nc.dram_tensor("ones8_d", [128, 2, 128], f8,
                             kind="ExternalInput")
    xf = nc.dram_tensor("xf", [C, NQ], f32, kind="ExternalInput")
    vnames = ("gn_w", "gn_b") if qfold else ("gn_w", "gn_b", "bq")
    vecs = {nm: nc.dram_tensor(nm, [C], f32, kind="ExternalInput")
            for nm in vnames}
    gmat_d = nc.dram_tensor("gmat_d", [128, GPT], f32, kind="ExternalInput")
    gmatT_d = nc.dram_tensor("gmatT_d", [GPT, 128], f32, kind="ExternalInput")
    out = nc.dram_tensor("out", [C, NQ], f32, kind="ExternalOutput")

    with tile.TileContext(nc) as tc, ExitStack() as ctx:
        consts = ctx.enter_context(tc.tile_pool(name="consts", bufs=1))

        x8t = [consts.tile([128, 2, N], f8, tag=f"x8_{pr}", name=f"x8_{pr}")
               for pr in range(CP)]
        w8t = {nm: [consts.tile([128, 2, C], f8, tag=f"{nm}{pr}",
                                name=f"{nm}{pr}") for pr in range(CP)]
               for nm in wnames}
        xft = [consts.tile([128, NQ], f32, tag=f"xf{ci}", name=f"xf{ci}")
               for ci in range(CT)]
        ones8 = consts.tile([128, 2, 128], f8, tag="ones8", name="ones8")
        vt = {}
        for nm in vnames:
            vt[nm] = consts.tile([128, CT], f32, tag=nm, name=nm)
        cstage = ctx.enter_context(tc.tile_pool(name="cstage", bufs=1))
        gm_st = cstage.tile([128, GPT], f32, tag="c1", name="gm_st")
        gmT_st = cstage.tile([GPT, 128], f32, tag="c2", name="gmT_st")

        # ---- DMA order: tiny constants first, then x8 (3 queues), weights,
        # residual ----
        nc.sync.dma_start(out=gm_st, in_=gmat_d[:, :])
        nc.gpsimd.dma_start(out=gmT_st, in_=gmatT_d[:, :])
        nc.scalar.dma_start(out=ones8, in_=ones8_d[:, :, :])
        for nm in vnames:
            nc.scalar.dma_start(
                out=vt[nm], in_=vecs[nm].rearrange("(c p) -> p c", p=128))

        warm_cm = tc.tile_pool(name="pp_warm", bufs=1, space="PSUM")
        pp_warm = warm_cm.__enter__()
        warm_ps = pp_warm.tile([128, 512], f32, tag="warm", name="warm")
        # dense burst first: the HAM clock-gate opens only after ~3.4us of
        # SUSTAINED PE activity; isolated blips never reach 2.4 GHz
        junk8 = consts.tile([128, 2, 512], f8, tag="junk8", name="junk8")
        nc.vector.memset(junk8, 0.25)

        def junk_mm(n):
            for _ in range(n):
                nc.tensor.matmul(
                    out=warm_ps, lhsT=junk8[:, :, :128], rhs=junk8,
                    start=True, stop=True, perf_mode=DR,
                    skip_group_check=True)

        junk_mm(40)
        dma_engs = (nc.sync, nc.gpsimd, nc.scalar)
        for ch in range(NC8):
            ns = slice(ch * 512, (ch + 1) * 512)
            for pr in range(CP):
                eng = dma_engs[(ch * CP + pr) % 3]
                eng.dma_start(out=x8t[pr][:, :, ns], in_=x8[pr, :, :, ns])
        for j, nm in enumerate(wnames):
            for pr in range(CP):
                eng = dma_engs[(j * CP + pr) % 3]
                eng.dma_start(out=w8t[nm][pr], in_=w8[nm][pr, :, :, :])
        for ci in range(CT):
            dma_engs[ci % 3].dma_start(out=xft[ci],
                                       in_=xf[ci * 128:(ci + 1) * 128, :])

        eps_t = consts.tile([128, 1], f32, tag="eps", name="eps")
        nc.vector.memset(eps_t, EPS)
        esh_t = consts.tile([128, 1], f32, tag="esh", name="esh")
        nc.vector.memset(esh_t, ESHIFT)
        zero_t = consts.tile([128, 1], f32, tag="zero", name="zero")
        nc.vector.memset(zero_t, 0.0)

        # ---- GroupNorm stats on fp8 X (1024-col batches, as DMAs land) ----
        sc_all = consts.tile([128, CT], f32, tag="sc_all", name="sc_all")
        bi_all = consts.tile([128, CT], f32, tag="bi_all", name="bi_all")
        with tc.tile_pool(name="gn_stats", bufs=1) as gstats, \
             tc.tile_pool(name="pp_gn", bufs=2, space="PSUM") as pp_gn:
            stats = [gstats.tile([128, NC8, 6], f32, tag=f"bnst{ci}",
                                 name=f"bnst{ci}") for ci in range(CT)]
            for ch in range(NC8):
                ns = slice(ch * 512, (ch + 1) * 512)
                for ci in range(CT):
                    nc.vector.bn_stats(out=stats[ci][:, ch, :],
                                       in_=x8t[ci // 2][:, ci % 2, ns])
                # junk blip on this chunk's arrival: holds the HAM
                # busy-window open through the stats phase
                nc.tensor.matmul(
                    out=warm_ps, lhsT=x8t[0][:, :, ch * 512:ch * 512 + 128],
                    rhs=x8t[0][:, :, ns], start=True, stop=True, perf_mode=DR,
                    skip_group_check=True)
            # f32r constants for the group-reduce matmuls
            gmat = consts.tile([128, GPT], f32r, tag="gmat", name="gmat")
            nc.vector.tensor_copy(out=gmat, in_=gm_st)
            gmatT = consts.tile([GPT, 128], f32r, tag="gmatT", name="gmatT")
            nc.vector.tensor_copy(out=gmatT, in_=gmT_st)
            rowst_all = gstats.tile([128, CT, 2], f32r, tag="rowst",
                                    name="rowst")
            for ci in range(CT):
                mv = gstats.tile([128, 2], f32, tag=f"mv{ci}", name=f"mv{ci}")
                nc.vector.bn_aggr(out=mv, in_=stats[ci])
                nc.vector.tensor_copy(out=rowst_all[:, ci, 0:1], in_=mv[:, 0:1])
                m2 = gstats.tile([128, 1], f32, tag=f"m2{ci}", name=f"m2{ci}")
                nc.vector.tensor_mul(out=m2, in0=mv[:, 0:1], in1=mv[:, 0:1])
                nc.vector.tensor_add(out=rowst_all[:, ci, 1:2],
                                     in0=mv[:, 1:2], in1=m2)
            # group-reduce 128 rows -> 8 groups -> broadcast back
            gps = pp_gn.tile([GPT, CT, 2], f32, tag="gps", name="gps")
            nc.tensor.matmul(out=gps, lhsT=gmat,
                             rhs=rowst_all.rearrange("p c two -> p (c two)"),
                             start=True, stop=True)
            gsb = gstats.tile([GPT, CT * 2], f32r, tag="gsb", name="gsb")
            nc.vector.tensor_copy(out=gsb,
                                  in_=gps.rearrange("g c two -> g (c two)"))
            bps = pp_gn.tile([128, CT, 2], f32, tag="bps", name="bps")
            nc.tensor.matmul(out=bps, lhsT=gmatT, rhs=gsb,
                             start=True, stop=True)
            junk_mm(12)  # keep the PE warm through the sc/bi vector chain
            gstat = gstats.tile([128, CT, 2], f32, tag="gstat", name="gstat")
            nc.scalar.mul(out=gstat, in_=bps, mul=1.0 / GSZ)
            means = gstat[:, :, 0:1].rearrange("p c one -> p (c one)")
            m2s = gstat[:, :, 1:2].rearrange("p c one -> p (c one)")
            var = gstats.tile([128, CT], f32, tag="var", name="var")
            mm_ = gstats.tile([128, CT], f32, tag="mm_", name="mm_")
            nc.vector.tensor_mul(out=mm_, in0=means, in1=means)
            nc.vector.tensor_sub(out=var, in0=m2s, in1=mm_)
            nc.scalar.activation(out=var, in_=var, func=AF.Sqrt,
                                 bias=eps_t, scale=1.0)
            rstd = gstats.tile([128, CT], f32, tag="rstd", name="rstd")
            nc.vector.reciprocal(out=rstd, in_=var)
            nc.vector.tensor_mul(out=sc_all, in0=rstd, in1=vt["gn_w"])
            msc = gstats.tile([128, CT], f32, tag="msc", name="msc")
            nc.vector.tensor_mul(out=msc, in0=means, in1=sc_all)
            nc.vector.tensor_sub(out=bi_all, in0=vt["gn_b"], in1=msc)

        # ---- hn8 + T/VP (+Q) projections (all DoubleRow fp8) ----
        hn8 = [consts.tile([128, 2, N], f8, tag=f"hn8_{pr}", name=f"hn8_{pr}")
               for pr in range(CP)]
        t8 = [consts.tile([128, 2, N], f8, tag=f"t8_{pr}", name=f"t8_{pr}")
              for pr in range(CP)]
        vp8 = [consts.tile([128, 2, C], f8, tag=f"vp8_{p}", name=f"vp8_{p}")
               for p in range(NTP)]
        if not qfold:
            q8 = [consts.tile([128, 2, NQ], f8, tag=f"q8_{pr}",
                              name=f"q8_{pr}") for pr in range(CP)]

        with tc.tile_pool(name="pp_proj", bufs=6, space="PSUM") as pp_proj:
            for ch in range(NC8):
                ns = slice(ch * 512, (ch + 1) * 512)
                # hn8 on the Pool engine (SBUF->SBUF only there)
                for ci in range(CT):
                    pr, i = ci // 2, ci % 2
                    nc.gpsimd.tensor_scalar(
                        out=hn8[pr][:, i, ns], in0=x8t[pr][:, i, ns],
                        scalar1=sc_all[:, ci:ci + 1],
                        scalar2=bi_all[:, ci:ci + 1],
                        op0=OP.mult, op1=OP.add)
                # T chunk: T = A^T @ Hn  (A = wk^T wq, host-folded)
                for co in range(CT):
                    ps = pp_proj.tile([128, 512], f32, tag="pps", name="t_ps")
                    for pr in range(CP):
                        nc.tensor.matmul(
                            out=ps,
                            lhsT=w8t["a8"][pr][:, :, co * 128:(co + 1) * 128],
                            rhs=hn8[pr][:, :, ns],
                            start=(pr == 0), stop=(pr == CP - 1), perf_mode=DR)
                    if co % 2:
                        nc.vector.tensor_copy(out=t8[co // 2][:, co % 2, ns],
                                              in_=ps)
                    else:
                        nc.scalar.activation(out=t8[co // 2][:, co % 2, ns],
                                             in_=ps, func=AF.Identity,
                                             bias=zero_t, scale=1.0)
                # VP chunk: 4 key tiles [k 128, c_out 512] of wpv @ Hn
                for nt4 in range(4):
                    nt = ch * 4 + nt4
                    ps = pp_proj.tile([128, 512], f32, tag="pps", name="v_ps")
                    for pr in range(CP):
                        nc.tensor.matmul(
                            out=ps,
                            lhsT=hn8[pr][:, :, nt * 128:(nt + 1) * 128],
                            rhs=w8t["wpv8"][pr],
                            start=(pr == 0), stop=(pr == CP - 1), perf_mode=DR)
                    if nt4 % 2:
                        nc.vector.tensor_copy(out=vp8[nt // 2][:, nt % 2, :],
                                              in_=ps)
                    else:
                        nc.scalar.activation(out=vp8[nt // 2][:, nt % 2, :],
                                             in_=ps, func=AF.Identity,
                                             bias=zero_t, scale=1.0)
                if not qfold and ch < QC:
                    for co in range(CT):
                        ps = pp_proj.tile([128, 512], f32, tag="pps",
                                          name="q_ps")
                        for pr in range(CP):
                            nc.tensor.matmul(
                                out=ps,
                                lhsT=w8t["wq8"][pr][:, :,
                                                    co * 128:(co + 1) * 128],
                                rhs=hn8[pr][:, :, ns],
                                start=(pr == 0), stop=(pr == CP - 1),
                                perf_mode=DR)
                        if co % 2:
                            nc.vector.tensor_scalar_add(
                                out=q8[co // 2][:, co % 2, ns], in0=ps,
                                scalar1=vt["bq"][:, co:co + 1])
                        else:
                            nc.scalar.activation(
                                out=q8[co // 2][:, co % 2, ns], in_=ps,
                                func=AF.Identity,
                                bias=vt["bq"][:, co:co + 1], scale=1.0)

        warm_cm.__exit__(None, None, None)
        qsrc = hn8 if qfold else q8

        # ---- attention ----
        with tc.tile_pool(name="es_pool", bufs=1) as es_pool, \
             tc.tile_pool(name="work", bufs=2) as work, \
             tc.tile_pool(name="pp_s", bufs=2, space="PSUM") as pp_s, \
             tc.tile_pool(name="pp_acc", bufs=1, space="PSUM") as pp_acc:
            es8 = [es_pool.tile([128, 2, 512], f8, tag=f"es{p}",
                                name=f"es{p}") for p in range(NTP)]
            for qc in range(QC):
                qs = slice(qc * 512, (qc + 1) * 512)
                acc_ps = [pp_acc.tile([128, 512], f32, tag=f"acc{ct}",
                                      name=f"acc{ct}") for ct in range(CT)]
                for ktp in range(NTP):
                    s_ps = pp_s.tile([128, 2, 512], f32, tag="s_ps",
                                     name="s_ps")
                    for i in range(2):
                        kt = 2 * ktp + i
                        for pr in range(CP):
                            nc.tensor.matmul(
                                out=s_ps[:, i, :],
                                lhsT=t8[pr][:, :, kt * 128:(kt + 1) * 128],
                                rhs=qsrc[pr][:, :, qs],
                                start=(pr == 0), stop=(pr == CP - 1),
                                perf_mode=DR)
                    nc.scalar.activation(out=es8[ktp], in_=s_ps, func=AF.Exp,
                                         scale=SCALE, bias=esh_t)
                    for ct in range(CT):
                        nc.tensor.matmul(
                            out=acc_ps[ct],
                            lhsT=vp8[ktp][:, :, ct * 128:(ct + 1) * 128],
                            rhs=es8[ktp],
                            start=(ktp == 0), stop=(ktp == NTP - 1),
                            perf_mode=DR)

                # sums post-pass into a recycled S slot (all rows identical)
                sums_ps = pp_s.tile([128, 2, 512], f32, tag="s_ps",
                                    name="sums")
                for ktp in range(NTP):
                    nc.tensor.matmul(
                        out=sums_ps[:, 0, :], lhsT=ones8, rhs=es8[ktp],
                        start=(ktp == 0), stop=(ktp == NTP - 1), perf_mode=DR)
                # inv = exp(-ln(sums)) on ACT: frees the psum slot in ~0.7us
                lns = work.tile([128, 512], f32, tag="lns", name="lns")
                nc.scalar.activation(out=lns, in_=sums_ps[:, 0, :],
                                     func=AF.Ln, bias=zero_t, scale=1.0)
                inv = work.tile([128, 512], f32, tag="inv", name="inv")
                nc.scalar.activation(out=inv, in_=lns, func=AF.Exp,
                                     bias=zero_t, scale=-1.0)
                for ct in range(CT):
                    ot = work.tile([128, 512], f32, tag="ot", name="ot",
                                   bufs=3)
                    nc.vector.tensor_mul(out=ot, in0=acc_ps[ct], in1=inv)
                    ot2 = work.tile([128, 512], f32, tag="ot2", name="ot2",
                                    bufs=3)
                    nc.gpsimd.tensor_add(out=ot2, in0=ot, in1=xft[ct][:, qs])
                    dma_engs[ct % 3].dma_start(
                        out=out[ct * 128:(ct + 1) * 128, qs], in_=ot2)

    nc.compile()
    return nc


def _get_nc(qfold=True):
    key = ("nc", qfold)
    if key not in _CACHE:
        _CACHE[key] = _build(qfold)
    return _CACHE[key]


def _pair8(a):
    """[C, F] f32 -> fp8 DoubleRow pair layout [CP, 128, 2, F]."""
    a8 = np.clip(a, -240.0, 240.0).astype(F8NP)
    return np.ascontiguousarray(
        a8.reshape(CP, 2, 128, a.shape[1]).transpose(0, 2, 1, 3))


def _prep_in_maps(X, gn_w, gn_b, wq, bq, wk, bk, wv, bv, wp, bp, qfold):
    f = lambda a: np.ascontiguousarray(np.asarray(a, dtype=np.float32))
    X = f(X)
    gn_w, gn_b, bq, bk, bv, bp = map(f, (gn_w, gn_b, bq, bk, bv, bp))
    wq, wk, wv, wp = map(f, (wq, wk, wv, wp))

    Xf = X.reshape(B, C, N)
    bpe = wp @ bv + bp  # bv folded through proj_out (softmax rows sum to 1)
    wpv = (wp.astype(np.float64) @ wv.astype(np.float64)).astype(np.float32)
    A = (wk.astype(np.float64).T @ wq.astype(np.float64)).astype(np.float32)
    w8 = {"a8": _pair8(A), "wpv8": _pair8(np.ascontiguousarray(wpv.T))}
    if not qfold:
        # general-bias path: separate Q projection, S against wk^T directly
        w8["a8"] = _pair8(np.ascontiguousarray(wk.T))
        w8["wq8"] = _pair8(np.ascontiguousarray(wq.T))

    gmat = np.zeros((128, GPT), np.float32)
    for g in range(GPT):
        gmat[g * GSZ:(g + 1) * GSZ, g] = 1.0
    gmatT = np.ascontiguousarray(gmat.T)
    ones8 = np.ones((128, 2, 128), F8NP)

    in_maps = []
    for core in range(8):
        bi, half = core // 2, core % 2
        x8p = _pair8(Xf[bi])
        if half:
            # swap key halves so queries are always columns 0..NQ
            x8p = np.ascontiguousarray(
                np.concatenate((x8p[..., NQ:], x8p[..., :NQ]), axis=-1))
        m = {
            "x8": x8p,
            "xf": np.ascontiguousarray(
                Xf[bi][:, half * NQ:(half + 1) * NQ] + bpe[:, None]),
            **w8,
            "ones8_d": ones8,
            "gn_w": gn_w, "gn_b": gn_b,
            "gmat_d": gmat, "gmatT_d": gmatT,
        }
        if not qfold:
            m["bq"] = bq
        in_maps.append(m)
    return in_maps


_last_in_maps = None


def kernel(X, gn_w, gn_b, wq, bq, wk, bk, wv, bv, wp, bp):
    from concourse.bass_utils import run_bass_kernel_spmd

    global _last_in_maps
    qfold = not np.any(np.asarray(bq))
    in_maps = _prep_in_maps(X, gn_w, gn_b, wq, bq, wk, bk, wv, bv, wp, bp,
                            qfold)
    _last_in_maps = in_maps
    nc = _get_nc(qfold)
    res = run_bass_kernel_spmd(nc, in_maps, list(range(8)))
    out = np.empty((B, C, N), np.float32)
    for core in range(8):
        bi, half = core // 2, core % 2
        out[bi][:, half * NQ:(half + 1) * NQ] = res.results[core]["out"]
    return out.reshape(B, C, H, W)
